# revision 20
# baseline (speedup 1.0000x reference)
"""GATConv (nn_GATConv_45595372814934) Trainium2 Bass kernel, 8 NeuronCores.

kernel(**inputs) -> [100000, 1, 64] float32.

Strategy (graph/edge parallelism):
- Node/edge shard: core c owns nodes [12500c, 12500(c+1)) and their 16
  out-edges each (src is repeat(arange(N), 16), so edges are contiguous).
- Phase 1 (per core): support shard = x_c @ W' where W' = [W | W@a_dst |
  W@a_src], fp16 rows [support(64) | s_dst | s_src] -> AllGather into a
  full per-core fp16 table [100352, 66] in HBM.
- Phase 2 (per core): node n = s*128 + p (partition p); its 16 edges sit
  along the free dim.  One indirect DMA per PAIR of 128-node super-tiles
  gathers 4096 table rows (by dst) into SBUF; per-edge weight
  exp(lrelu(s_src + s_dst) - ln(deg)) via per-partition tensor_scalar +
  Exp activation with bias; weighted sum over the 16 edges is a vector
  multiply + strided reduce (no matmuls, no PSUM in phase 2).
"""

import os
import sys

sys.path.insert(0, "/opt/trn_rl_repo")

import numpy as np

import concourse.bacc as bacc
import concourse.bass as bass
import concourse.mybir as mybir
import concourse.tile as tile
from concourse.bass import AP
from concourse import bass_utils

F32 = mybir.dt.float32
F16 = mybir.dt.float16
I32 = mybir.dt.int32

N_NODES = 100000
IN_CH = 256
C = 64
DEG = 16
NEG_SLOPE = 0.2
NCORES = 8
NPC = N_NODES // NCORES          # 12500 real nodes per core
NPAD = -(-NPC // 128) * 128      # 12544
SUP = NPAD // 128                # 98 super-tiles
PAIRS = SUP // 2                 # 49 gather iterations (2 super-tiles each)
NROWS = NCORES * NPAD            # table rows
TW = 66                          # table row: support(64) | s_dst | s_src

LAST_EXEC_NS = None
_CACHED_NC = None


def _mkap(base: AP, extra_off: int, dims) -> AP:
    return AP(base.tensor, base.offset + extra_off,
              [list(base.ap[0])] + [list(d) for d in dims])


def _build_nc():
    nc = bacc.Bacc("TRN2", target_bir_lowering=False, debug=False,
                   num_devices=NCORES, num_swdge_queues=4)

    xT_d = nc.dram_tensor("xT", [IN_CH, NPAD], F16, kind="ExternalInput")
    dstT_d = nc.dram_tensor("dstT", [128, SUP * DEG], I32, kind="ExternalInput")
    nlnd_d = nc.dram_tensor("nlnd", [128, SUP], F32, kind="ExternalInput")
    wp_d = nc.dram_tensor("wp", [IN_CH, TW], F16, kind="ExternalInput")
    out_d = nc.dram_tensor("out", [NPAD, C], F32, kind="ExternalOutput")

    from concourse.replica_groups import maybe_share_collective_output_space
    aspace = maybe_share_collective_output_space(
        "AllGather", [list(range(NCORES))])
    shard_d = nc.dram_tensor("shard", [NPAD, TW], F16, kind="Internal")
    table_d = nc.dram_tensor("table", [NROWS, TW], F16, kind="Internal",
                             addr_space=aspace)

    dst_sb = nc.alloc_sbuf_tensor("dst_sb", [128, SUP * DEG], I32)
    ssrc_sb = nc.alloc_sbuf_tensor("ssrc_sb", [128, SUP], F32)
    nlnd_sb = nc.alloc_sbuf_tensor("nlnd_sb", [128, SUP], F32)
    wp_sb = nc.alloc_sbuf_tensor("wp_sb", [128, 2 * TW], F16)

    with tile.TileContext(nc) as tc:
        with (
            tc.tile_pool(name="xp", bufs=3) as xp,
            tc.tile_pool(name="stp", bufs=3) as stp,
            tc.tile_pool(name="gp", bufs=2) as gp,
            tc.tile_pool(name="pp", bufs=2) as pp,
            tc.tile_pool(name="sp", bufs=2) as sp,
            tc.tile_pool(name="obp", bufs=2) as obp,
            tc.tile_pool(name="ps1", bufs=2, space="PSUM") as ps1,
        ):
            nc.sync.dma_start(dst_sb.ap(), dstT_d.ap())
            nc.sync.dma_start(nlnd_sb.ap(), nlnd_d.ap())
            nc.sync.dma_start(
                wp_sb.ap(), wp_d.ap().rearrange("(a p) c -> p a c", p=128))
            wp3 = wp_sb.ap().rearrange("p (a c) -> p a c", c=TW)

            # phase 1: support table shard
            xT3 = xT_d.ap().rearrange("(a p) n -> p a n", p=128)
            for s in range(SUP):
                xt = xp.tile([128, 2, 128], F16, tag="xt")
                nc.sync.dma_start(xt[:], xT3[:, :, 128 * s:128 * (s + 1)])
                ps = ps1.tile([128, TW], F32, tag="ps1")
                nc.tensor.matmul(out=ps[:], lhsT=xt[:, 0, :], rhs=wp3[:, 0, :],
                                 start=True, stop=False)
                nc.tensor.matmul(out=ps[:], lhsT=xt[:, 1, :], rhs=wp3[:, 1, :],
                                 start=False, stop=True)
                st = stp.tile([128, TW], F16, tag="st")
                nc.scalar.copy(st[:], ps[:])
                nc.vector.tensor_copy(ssrc_sb.ap()[:, s:s + 1],
                                      ps[:, TW - 1:TW])
                nc.sync.dma_start(shard_d.ap()[128 * s:128 * (s + 1), :], st[:])

            nc.gpsimd.collective_compute(
                "AllGather", mybir.AluOpType.bypass,
                replica_groups=[list(range(NCORES))],
                ins=[shard_d.ap()], outs=[table_d.ap()])

            # phase 2: gather + per-edge weights + weighted segment sum
            out3 = out_d.ap().rearrange("(s p) c -> p s c", p=128)
            for s in range(SUP):
                G = gp.tile([128, DEG, TW], F16, tag="G")
                for t in range(DEG):
                    gi = nc.gpsimd.indirect_dma_start(
                        out=G[:, t, :], out_offset=None,
                        in_=table_d.ap(),
                        in_offset=bass.IndirectOffsetOnAxis(
                            ap=dst_sb.ap()[:, DEG * s + t:DEG * s + t + 1],
                            axis=0))
                    if t % 4:
                        gi.queue = f"qPoolDynamic{t % 4}"

                g_sd = _mkap(G[:], C, [[TW, DEG]])
                sc = sp.tile([128, DEG], F32, tag="sc")
                nc.vector.tensor_scalar(
                    out=sc[:], in0=g_sd,
                    scalar1=ssrc_sb.ap()[:, s:s + 1], scalar2=None,
                    op0=mybir.AluOpType.add)
                lr = sp.tile([128, DEG], F32, tag="lr")
                nc.vector.scalar_tensor_tensor(
                    out=lr[:], in0=sc[:], scalar=NEG_SLOPE, in1=sc[:],
                    op0=mybir.AluOpType.mult, op1=mybir.AluOpType.max)
                wt = sp.tile([128, DEG], F32, tag="wt")
                nc.scalar.activation(
                    wt[:], lr[:], mybir.ActivationFunctionType.Exp,
                    bias=nlnd_sb.ap()[:, s:s + 1])

                prod = pp.tile([128, DEG, C], F32, tag="prod")
                g_sup = _mkap(G[:], 0, [[TW, DEG], [1, C]])
                nc.vector.tensor_tensor(
                    out=prod[:], in0=g_sup,
                    in1=wt[:].to_broadcast([128, DEG, C]),
                    op=mybir.AluOpType.mult)

                ob = obp.tile([128, C], F32, tag="ob")
                red_in = _mkap(prod[:], 0, [[1, C], [C, DEG]])
                nc.vector.tensor_reduce(
                    out=ob[:], in_=red_in,
                    axis=mybir.AxisListType.X, op=mybir.AluOpType.add)
                nc.sync.dma_start(out3[:, s:s + 1, :], ob[:])

    nc.compile()
    return nc


def _host_prep(x, dst, adj_values, weight, attention):
    dst = np.asarray(dst)
    dst_rows = ((dst // NPC) * NPAD + dst % NPC).astype(np.int32)

    weight = np.asarray(weight, np.float32)
    att = np.asarray(attention, np.float32).reshape(2 * C)
    a_src, a_dst = att[:C], att[C:]
    wp = np.empty((IN_CH, TW), np.float32)
    wp[:, :C] = weight
    wp[:, C] = weight @ a_dst
    wp[:, C + 1] = weight @ a_src
    wp = np.ascontiguousarray(wp.astype(np.float16))

    adj = np.asarray(adj_values, np.float32).reshape(N_NODES, DEG)
    deg = adj.sum(axis=1)

    in_maps = []
    for c in range(NCORES):
        xT = np.zeros((IN_CH, NPAD), np.float16)
        xT[:, :NPC] = np.asarray(x[c * NPC:(c + 1) * NPC], np.float32).T
        nlnd = np.full((NPAD,), -np.log(np.float32(DEG)), np.float32)
        nlnd[:NPC] = -np.log(deg[c * NPC:(c + 1) * NPC])
        nlnd = np.ascontiguousarray(nlnd.reshape(SUP, 128).T)
        dr = np.zeros((NPAD, DEG), np.int32)
        dr[:NPC] = dst_rows[c * NPC * DEG:(c + 1) * NPC * DEG].reshape(NPC, DEG)
        dstT = (dr.reshape(SUP, 128, DEG)
                  .transpose(1, 0, 2)
                  .reshape(128, SUP * DEG))
        in_maps.append({
            "xT": xT,
            "dstT": np.ascontiguousarray(dstT),
            "nlnd": nlnd,
            "wp": wp,
        })
    return in_maps


def _numpy_fallback(x, edge_index, adj_values, weight, attention):
    N = x.shape[0]
    x = np.asarray(x, np.float32)
    support = (x @ np.asarray(weight, np.float32)).reshape(N, 1, C)
    src = np.asarray(edge_index[0])
    dst = np.asarray(edge_index[1])
    att = np.asarray(attention, np.float32).reshape(1, 1, 2 * C)
    a_src, a_dst = att[0, :, :C], att[0, :, C:]
    s_src = np.einsum('nhc,hc->nh', support, a_src)
    s_dst = np.einsum('nhc,hc->nh', support, a_dst)
    z = s_src[src] + s_dst[dst]
    edge_e = np.exp(np.where(z >= 0, z, NEG_SLOPE * z))
    deg = np.zeros(N, np.float32)
    np.add.at(deg, src, np.asarray(adj_values, np.float32))
    edge_e = edge_e / deg[src][:, None]
    out = np.zeros((N, 1, C), np.float32)
    np.add.at(out, src, edge_e[:, :, None] * support[dst])
    return out.astype(np.float32)


def kernel(x, edge_index, adj_values, weight, attention):
    global LAST_EXEC_NS, _CACHED_NC
    x = np.asarray(x)
    edge_index = np.asarray(edge_index)
    src = edge_index[0]

    expected_src = np.repeat(
        np.arange(N_NODES, dtype=src.dtype), DEG)
    if x.shape[0] != N_NODES or not np.array_equal(src, expected_src):
        # unexpected structure: fall back to a host reference implementation
        return _numpy_fallback(x, edge_index, adj_values, weight, attention)

    if _CACHED_NC is None:
        _CACHED_NC = _build_nc()
    nc = _CACHED_NC

    in_maps = _host_prep(x, edge_index[1], adj_values, weight, attention)

    trace = os.environ.get("GAT_BASS_TRACE", "") == "1"
    kwargs = {}
    if trace:
        try:
            import prof_hook
            prof_hook.install()
        except Exception:
            trace = False
    res = bass_utils.run_bass_kernel_spmd(
        nc, in_maps, core_ids=list(range(NCORES)), trace=trace)
    LAST_EXEC_NS = res.exec_time_ns

    parts = [res.results[c]["out"][:NPC] for c in range(NCORES)]
    out = np.concatenate(parts, 0).reshape(N_NODES, 1, C)
    return np.ascontiguousarray(out.astype(np.float32))


# revision 21
# speedup vs baseline: 1.0031x; 1.0031x over previous
"""GATConv (nn_GATConv_45595372814934) Trainium2 Bass kernel, 8 NeuronCores.

kernel(**inputs) -> [100000, 1, 64] float32.

Strategy (graph/edge parallelism):
- Node/edge shard: core c owns nodes [12500c, 12500(c+1)) and their 16
  out-edges each (src is repeat(arange(N), 16), so edges are contiguous).
- Phase 1 (per core): support shard = x_c @ W' where W' = [W | W@a_dst |
  W@a_src], fp16 rows [support(64) | s_dst | s_src] -> AllGather into a
  full per-core fp16 table [100352, 66] in HBM.
- Phase 2 (per core): node n = s*128 + p (partition p); its 16 edges sit
  along the free dim.  One indirect DMA per PAIR of 128-node super-tiles
  gathers 4096 table rows (by dst) into SBUF; per-edge weight
  exp(lrelu(s_src + s_dst) - ln(deg)) via per-partition tensor_scalar +
  Exp activation with bias; weighted sum over the 16 edges is a vector
  multiply + strided reduce (no matmuls, no PSUM in phase 2).
"""

import os
import sys

sys.path.insert(0, "/opt/trn_rl_repo")

import numpy as np

import concourse.bacc as bacc
import concourse.bass as bass
import concourse.mybir as mybir
import concourse.tile as tile
from concourse.bass import AP
from concourse import bass_utils

F32 = mybir.dt.float32
F16 = mybir.dt.float16
I32 = mybir.dt.int32

N_NODES = 100000
IN_CH = 256
C = 64
DEG = 16
NEG_SLOPE = 0.2
NCORES = 8
NPC = N_NODES // NCORES          # 12500 real nodes per core
NPAD = -(-NPC // 128) * 128      # 12544
SUP = NPAD // 128                # 98 super-tiles
PAIRS = SUP // 2                 # 49 gather iterations (2 super-tiles each)
NROWS = NCORES * NPAD            # table rows
TW = 66                          # table row: support(64) | s_dst | s_src

LAST_EXEC_NS = None
_CACHED_NC = None


def _mkap(base: AP, extra_off: int, dims) -> AP:
    return AP(base.tensor, base.offset + extra_off,
              [list(base.ap[0])] + [list(d) for d in dims])


def _build_nc():
    nc = bacc.Bacc("TRN2", target_bir_lowering=False, debug=False,
                   num_devices=NCORES, num_swdge_queues=4)

    xT_d = nc.dram_tensor("xT", [IN_CH, NPAD], F16, kind="ExternalInput")
    dstT_d = nc.dram_tensor("dstT", [128, SUP * DEG], I32, kind="ExternalInput")
    nlnd_d = nc.dram_tensor("nlnd", [128, SUP], F32, kind="ExternalInput")
    wp_d = nc.dram_tensor("wp", [IN_CH, TW], F16, kind="ExternalInput")
    out_d = nc.dram_tensor("out", [NPAD, C], F32, kind="ExternalOutput")

    from concourse.replica_groups import maybe_share_collective_output_space
    aspace = maybe_share_collective_output_space(
        "AllGather", [list(range(NCORES))])
    shard_d = nc.dram_tensor("shard", [NPAD, TW], F16, kind="Internal")
    table_d = nc.dram_tensor("table", [NROWS, TW], F16, kind="Internal",
                             addr_space=aspace)

    dst_sb = nc.alloc_sbuf_tensor("dst_sb", [128, SUP * DEG], I32)
    ssrc_sb = nc.alloc_sbuf_tensor("ssrc_sb", [128, SUP], F32)
    nlnd_sb = nc.alloc_sbuf_tensor("nlnd_sb", [128, SUP], F32)
    wp_sb = nc.alloc_sbuf_tensor("wp_sb", [128, 2 * TW], F16)

    with tile.TileContext(nc) as tc:
        with (
            tc.tile_pool(name="xp", bufs=3) as xp,
            tc.tile_pool(name="stp", bufs=3) as stp,
            tc.tile_pool(name="gp", bufs=4) as gp,
            tc.tile_pool(name="pp", bufs=3) as pp,
            tc.tile_pool(name="sp", bufs=3) as sp,
            tc.tile_pool(name="obp", bufs=3) as obp,
            tc.tile_pool(name="ps1", bufs=2, space="PSUM") as ps1,
        ):
            nc.sync.dma_start(dst_sb.ap(), dstT_d.ap())
            nc.sync.dma_start(nlnd_sb.ap(), nlnd_d.ap())
            nc.sync.dma_start(
                wp_sb.ap(), wp_d.ap().rearrange("(a p) c -> p a c", p=128))
            wp3 = wp_sb.ap().rearrange("p (a c) -> p a c", c=TW)

            # phase 1: support table shard
            xT3 = xT_d.ap().rearrange("(a p) n -> p a n", p=128)
            for s in range(SUP):
                xt = xp.tile([128, 2, 128], F16, tag="xt")
                nc.sync.dma_start(xt[:], xT3[:, :, 128 * s:128 * (s + 1)])
                ps = ps1.tile([128, TW], F32, tag="ps1")
                nc.tensor.matmul(out=ps[:], lhsT=xt[:, 0, :], rhs=wp3[:, 0, :],
                                 start=True, stop=False)
                nc.tensor.matmul(out=ps[:], lhsT=xt[:, 1, :], rhs=wp3[:, 1, :],
                                 start=False, stop=True)
                st = stp.tile([128, TW], F16, tag="st")
                nc.scalar.copy(st[:], ps[:])
                nc.vector.tensor_copy(ssrc_sb.ap()[:, s:s + 1],
                                      ps[:, TW - 1:TW])
                nc.sync.dma_start(shard_d.ap()[128 * s:128 * (s + 1), :], st[:])

            nc.gpsimd.collective_compute(
                "AllGather", mybir.AluOpType.bypass,
                replica_groups=[list(range(NCORES))],
                ins=[shard_d.ap()], outs=[table_d.ap()])

            # phase 2: gather + per-edge weights + weighted segment sum
            out3 = out_d.ap().rearrange("(s p) c -> p s c", p=128)
            for s in range(SUP):
                G = gp.tile([128, DEG, TW], F16, tag="G")
                for t in range(DEG):
                    gi = nc.gpsimd.indirect_dma_start(
                        out=G[:, t, :], out_offset=None,
                        in_=table_d.ap(),
                        in_offset=bass.IndirectOffsetOnAxis(
                            ap=dst_sb.ap()[:, DEG * s + t:DEG * s + t + 1],
                            axis=0))
                    if t % 4:
                        gi.queue = f"qPoolDynamic{t % 4}"

                g_sd = _mkap(G[:], C, [[TW, DEG]])
                sc = sp.tile([128, DEG], F32, tag="sc")
                nc.vector.tensor_scalar(
                    out=sc[:], in0=g_sd,
                    scalar1=ssrc_sb.ap()[:, s:s + 1], scalar2=None,
                    op0=mybir.AluOpType.add)
                lr = sp.tile([128, DEG], F32, tag="lr")
                nc.vector.scalar_tensor_tensor(
                    out=lr[:], in0=sc[:], scalar=NEG_SLOPE, in1=sc[:],
                    op0=mybir.AluOpType.mult, op1=mybir.AluOpType.max)
                wt = sp.tile([128, DEG], F32, tag="wt")
                nc.scalar.activation(
                    wt[:], lr[:], mybir.ActivationFunctionType.Exp,
                    bias=nlnd_sb.ap()[:, s:s + 1])

                prod = pp.tile([128, DEG, C], F32, tag="prod")
                g_sup = _mkap(G[:], 0, [[TW, DEG], [1, C]])
                nc.vector.tensor_tensor(
                    out=prod[:], in0=g_sup,
                    in1=wt[:].to_broadcast([128, DEG, C]),
                    op=mybir.AluOpType.mult)

                ob = obp.tile([128, C], F32, tag="ob")
                red_in = _mkap(prod[:], 0, [[1, C], [C, DEG]])
                nc.vector.tensor_reduce(
                    out=ob[:], in_=red_in,
                    axis=mybir.AxisListType.X, op=mybir.AluOpType.add)
                nc.sync.dma_start(out3[:, s:s + 1, :], ob[:])

    nc.compile()
    return nc


def _host_prep(x, dst, adj_values, weight, attention):
    dst = np.asarray(dst)
    dst_rows = ((dst // NPC) * NPAD + dst % NPC).astype(np.int32)

    weight = np.asarray(weight, np.float32)
    att = np.asarray(attention, np.float32).reshape(2 * C)
    a_src, a_dst = att[:C], att[C:]
    wp = np.empty((IN_CH, TW), np.float32)
    wp[:, :C] = weight
    wp[:, C] = weight @ a_dst
    wp[:, C + 1] = weight @ a_src
    wp = np.ascontiguousarray(wp.astype(np.float16))

    adj = np.asarray(adj_values, np.float32).reshape(N_NODES, DEG)
    deg = adj.sum(axis=1)

    in_maps = []
    for c in range(NCORES):
        xT = np.zeros((IN_CH, NPAD), np.float16)
        xT[:, :NPC] = np.asarray(x[c * NPC:(c + 1) * NPC], np.float32).T
        nlnd = np.full((NPAD,), -np.log(np.float32(DEG)), np.float32)
        nlnd[:NPC] = -np.log(deg[c * NPC:(c + 1) * NPC])
        nlnd = np.ascontiguousarray(nlnd.reshape(SUP, 128).T)
        dr = np.zeros((NPAD, DEG), np.int32)
        dr[:NPC] = dst_rows[c * NPC * DEG:(c + 1) * NPC * DEG].reshape(NPC, DEG)
        dstT = (dr.reshape(SUP, 128, DEG)
                  .transpose(1, 0, 2)
                  .reshape(128, SUP * DEG))
        in_maps.append({
            "xT": xT,
            "dstT": np.ascontiguousarray(dstT),
            "nlnd": nlnd,
            "wp": wp,
        })
    return in_maps


def _numpy_fallback(x, edge_index, adj_values, weight, attention):
    N = x.shape[0]
    x = np.asarray(x, np.float32)
    support = (x @ np.asarray(weight, np.float32)).reshape(N, 1, C)
    src = np.asarray(edge_index[0])
    dst = np.asarray(edge_index[1])
    att = np.asarray(attention, np.float32).reshape(1, 1, 2 * C)
    a_src, a_dst = att[0, :, :C], att[0, :, C:]
    s_src = np.einsum('nhc,hc->nh', support, a_src)
    s_dst = np.einsum('nhc,hc->nh', support, a_dst)
    z = s_src[src] + s_dst[dst]
    edge_e = np.exp(np.where(z >= 0, z, NEG_SLOPE * z))
    deg = np.zeros(N, np.float32)
    np.add.at(deg, src, np.asarray(adj_values, np.float32))
    edge_e = edge_e / deg[src][:, None]
    out = np.zeros((N, 1, C), np.float32)
    np.add.at(out, src, edge_e[:, :, None] * support[dst])
    return out.astype(np.float32)


def kernel(x, edge_index, adj_values, weight, attention):
    global LAST_EXEC_NS, _CACHED_NC
    x = np.asarray(x)
    edge_index = np.asarray(edge_index)
    src = edge_index[0]

    expected_src = np.repeat(
        np.arange(N_NODES, dtype=src.dtype), DEG)
    if x.shape[0] != N_NODES or not np.array_equal(src, expected_src):
        # unexpected structure: fall back to a host reference implementation
        return _numpy_fallback(x, edge_index, adj_values, weight, attention)

    if _CACHED_NC is None:
        _CACHED_NC = _build_nc()
    nc = _CACHED_NC

    in_maps = _host_prep(x, edge_index[1], adj_values, weight, attention)

    trace = os.environ.get("GAT_BASS_TRACE", "") == "1"
    kwargs = {}
    if trace:
        try:
            import prof_hook
            prof_hook.install()
        except Exception:
            trace = False
    res = bass_utils.run_bass_kernel_spmd(
        nc, in_maps, core_ids=list(range(NCORES)), trace=trace)
    LAST_EXEC_NS = res.exec_time_ns

    parts = [res.results[c]["out"][:NPC] for c in range(NCORES)]
    out = np.concatenate(parts, 0).reshape(N_NODES, 1, C)
    return np.ascontiguousarray(out.astype(np.float32))
